# revision 5
# baseline (speedup 1.0000x reference)
"""Beta-TCVAE loss kernel for Trainium2, 8 NeuronCores, data-parallel over rows.

Math (see reference): with elem[i,j,d] = A[j,d] + M2[i,d]*B[j,d] where
  A = -0.5*(zlv + log 2pi), B = -0.5/(exp(zlv)+tol), M2 = z_mean^2,
the loss collapses (log_pz cancels) to
  out = -(log_px - 5*mean_i log_qz[i] + 5*mean_i log_qz_prod[i])
  log_qz_prod[i] = D*(log S - log nm) + sum_d m[i,d],
      m[i,d] = max_j elem[i,j,d],  S = sum_{i,j,d} exp(elem - m[i,d])
  log_qz[i] = log S2 + m2[i] - log nm,
      R[i,j] = Asum[j] + sum_d M2[i,d]B[j,d],  m2[i] = max_j R,
      S2 = sum_{i,j} exp(R - m2[i])
  log_px = mean_i sum_p [t*log(xm+tol) + (1-t)*log(1-xm+tol)]

m[i,d] is computed EXACTLY on host: elem as a function of lv=zlv[j,d] is
strictly concave, so the discrete max over j lies at the sorted-lv values
bracketing the continuous argmax (u* solves x*u = (u+tol)^2).  Everything
O(N^2 D) / O(N PIX) runs on the device:
 - TensorE forms elem - m via K=128 matmuls (zero-padded weights carry
   M2 rows, a ones row, and a -m row), PSUM holds [128, 2048] (2 d's).
 - ScalarE does exp with fused accum (row sums), one instr per PSUM tile.
 - log_px: ScalarE Ln (x2) + VectorE sub + fused multiply-reduce.
Per-core partial sums return to host; final combination in float64.
"""

import math

import numpy as np

import concourse.bacc as bacc
import concourse.tile as tile
from concourse import mybir
from concourse.bass_utils import run_bass_kernel_spmd

F32 = mybir.dt.float32
AF = mybir.ActivationFunctionType
ALU = mybir.AluOpType

_TOL = 1e-7
DATASET_SIZE = 737280
N, D, PIX = 1024, 64, 12288
LOG_2PI = math.log(2.0 * math.pi)
LOG_NM = math.log(float(N * DATASET_SIZE))
NCORES = 8
ROWS = N // NCORES  # 128
CH = 2048
NCH = PIX // CH  # 6
NDBIG = D - 1  # d = 0..62 via the big zero-padded pack; d = 63 via K=3 tail
DPAIRS = D // 2  # 32 psum tiles, 2 d's each


def _build_program():
    nc = bacc.Bacc("TRN2", target_bir_lowering=False, debug=False)

    # ---- DRAM I/O (per core; SPMD over 8 cores) ----
    t_rows = nc.dram_tensor("t_rows", [ROWS, PIX], F32, kind="ExternalInput")
    xm_rows = nc.dram_tensor("xm_rows", [ROWS, PIX], F32, kind="ExternalInput")
    b1_lhsT = nc.dram_tensor("b1_lhsT", [128, NDBIG * 128], F32, kind="ExternalInput")
    b1_rhs = nc.dram_tensor("b1_rhs", [128, N], F32, kind="ExternalInput")
    b1_lhsT_t = nc.dram_tensor("b1_lhsT_t", [3, 128], F32, kind="ExternalInput")
    b1_rhs_t = nc.dram_tensor("b1_rhs_t", [3, N], F32, kind="ExternalInput")
    b2_lhsT = nc.dram_tensor("b2_lhsT", [D + 1, 128], F32, kind="ExternalInput")
    b2_rhs = nc.dram_tensor("b2_rhs", [D + 1, N], F32, kind="ExternalInput")

    u_parts_d = nc.dram_tensor("u_parts", [128, DPAIRS], F32, kind="ExternalOutput")
    negm2_d = nc.dram_tensor("negm2", [128, 1], F32, kind="ExternalOutput")
    u2_d = nc.dram_tensor("u2", [128, 1], F32, kind="ExternalOutput")
    l2sums_d = nc.dram_tensor("l2sums", [128, NCH], F32, kind="ExternalOutput")
    psums_d = nc.dram_tensor("psums", [128, NCH], F32, kind="ExternalOutput")

    with tile.TileContext(nc) as tc:
        with (
            tc.tile_pool(name="consts", bufs=1) as consts,
            tc.tile_pool(name="work", bufs=3) as work,
            tc.tile_pool(name="scr", bufs=2) as scr,
            tc.tile_pool(name="outs", bufs=1) as outs,
            tc.tile_pool(name="psum", bufs=2, space="PSUM") as psum,
        ):
            # resident small tensors
            b1_lhsT_s = consts.tile([128, NDBIG * 128], F32)
            nc.sync.dma_start(out=b1_lhsT_s, in_=b1_lhsT[:, :])
            b1_rhs_s = consts.tile([128, N], F32)
            nc.sync.dma_start(out=b1_rhs_s, in_=b1_rhs[:, :])
            b1_lhsT_t_s = consts.tile([3, 128], F32)
            nc.sync.dma_start(out=b1_lhsT_t_s, in_=b1_lhsT_t[:, :])
            b1_rhs_t_s = consts.tile([3, N], F32)
            nc.sync.dma_start(out=b1_rhs_t_s, in_=b1_rhs_t[:, :])
            b2_lhsT_s = consts.tile([D + 1, 128], F32)
            nc.sync.dma_start(out=b2_lhsT_s, in_=b2_lhsT[:, :])
            b2_rhs_s = consts.tile([D + 1, N], F32)
            nc.sync.dma_start(out=b2_rhs_s, in_=b2_rhs[:, :])

            zero_c = consts.tile([128, 1], F32, tag="zc")
            nc.vector.memset(zero_c, 0.0)
            tol_c = consts.tile([128, 1], F32, tag="tc")
            nc.vector.memset(tol_c, _TOL)
            onep_c = consts.tile([128, 1], F32, tag="oc")
            nc.vector.memset(onep_c, 1.0 + _TOL)

            u_parts_s = outs.tile([128, DPAIRS], F32)
            negm2_s = outs.tile([128, 1], F32)
            u2_s = outs.tile([128, 1], F32)
            l2sums_s = outs.tile([128, NCH], F32)
            psums_s = outs.tile([128, NCH], F32)

            # ---- B2: R = Asum[j] + sum_d M2[i,d] B[j,d]; m2, U2 ----
            r_ps = psum.tile([128, N], F32, tag="pt")
            for j0 in (0, 512):
                nc.tensor.matmul(
                    out=r_ps[:, j0 : j0 + 512],
                    lhsT=b2_lhsT_s,
                    rhs=b2_rhs_s[:, j0 : j0 + 512],
                    start=True,
                    stop=True,
                )
            nc.vector.tensor_reduce(
                out=negm2_s,
                in_=r_ps,
                axis=mybir.AxisListType.X,
                op=ALU.max,
                negate=True,
            )
            nc.scalar.activation(
                out=r_ps,
                in_=r_ps,
                func=AF.Exp,
                bias=negm2_s[:],
                scale=1.0,
                accum_out=u2_s,
            )
            nc.sync.dma_start(out=negm2_d[:, :], in_=negm2_s)
            nc.sync.dma_start(out=u2_d[:, :], in_=u2_s)

            # ---- B1: 32 psum tiles, each = elem - m for a pair of d's ----
            for k in range(DPAIRS):
                pt = psum.tile([128, 2 * N], F32, tag="pt")
                for half in range(2):
                    d = 2 * k + half
                    for j0 in (0, 512):
                        o = pt[:, half * N + j0 : half * N + j0 + 512]
                        if d < NDBIG:
                            nc.tensor.matmul(
                                out=o,
                                lhsT=b1_lhsT_s[:, d * 128 : (d + 1) * 128],
                                rhs=b1_rhs_s[:, j0 : j0 + 512],
                                start=True,
                                stop=True,
                            )
                        else:
                            nc.tensor.matmul(
                                out=o,
                                lhsT=b1_lhsT_t_s,
                                rhs=b1_rhs_t_s[:, j0 : j0 + 512],
                                start=True,
                                stop=True,
                            )
                nc.scalar.activation(
                    out=pt,
                    in_=pt,
                    func=AF.Exp,
                    bias=zero_c[:],
                    scale=1.0,
                    accum_out=u_parts_s[:, k : k + 1],
                )
            nc.sync.dma_start(out=u_parts_d[:, :], in_=u_parts_s)

            # ---- A: log_px partial sums ----
            for c in range(NCH):
                tt = work.tile([128, CH], F32, tag="tt")
                nc.sync.dma_start(out=tt, in_=t_rows[:, c * CH : (c + 1) * CH])
                xt = work.tile([128, CH], F32, tag="xt")
                nc.sync.dma_start(out=xt, in_=xm_rows[:, c * CH : (c + 1) * CH])
                l1 = work.tile([128, CH], F32, tag="l1")
                nc.scalar.activation(out=l1, in_=xt, func=AF.Ln, bias=tol_c[:], scale=1.0)
                l2 = work.tile([128, CH], F32, tag="l2")
                nc.scalar.activation(
                    out=l2,
                    in_=xt,
                    func=AF.Ln,
                    bias=onep_c[:],
                    scale=-1.0,
                    accum_out=l2sums_s[:, c : c + 1],
                )
                nc.vector.tensor_sub(out=l1, in0=l1, in1=l2)
                ps = scr.tile([128, CH], F32, tag="ps")
                nc.vector.scalar_tensor_tensor(
                    out=ps,
                    in0=tt,
                    scalar=1.0,
                    in1=l1,
                    op0=ALU.mult,
                    op1=ALU.mult,
                    accum_out=psums_s[:, c : c + 1],
                )
            nc.sync.dma_start(out=l2sums_d[:, :], in_=l2sums_s)
            nc.sync.dma_start(out=psums_d[:, :], in_=psums_s)

    nc.compile()
    return nc


_NC_CACHE = None


def _get_program():
    global _NC_CACHE
    if _NC_CACHE is None:
        _NC_CACHE = _build_program()
    return _NC_CACHE


def host_prep(z_mean, z_log_var):
    """A, B, M2 [N,D] f32 and the exact per-(i,d) max m [N,D] f32."""
    zlv = np.asarray(z_log_var, dtype=np.float32)
    M2 = np.square(np.asarray(z_mean, dtype=np.float32))
    ez = np.exp(zlv)
    B = (-0.5 / (ez + _TOL)).astype(np.float32)
    A = (-0.5 * (zlv + LOG_2PI)).astype(np.float32)

    x = M2.astype(np.float64)
    tol = float(_TOL)
    disc = np.maximum((x - 2 * tol) ** 2 - 4 * tol * tol, 0.0)
    ustar = ((x - 2 * tol) + np.sqrt(disc)) / 2.0
    with np.errstate(divide="ignore"):
        lvstar = np.where(x <= 4 * tol, -np.inf, np.log(np.maximum(ustar, 1e-300)))

    m = np.empty((N, D), dtype=np.float32)
    for d in range(D):
        s = np.sort(zlv[:, d].astype(np.float64))
        pos = np.searchsorted(s, lvstar[:, d])
        cands = np.stack([np.clip(pos + k, 0, N - 1) for k in (-2, -1, 0, 1)], axis=1)
        lv_c = s[cands].astype(np.float32)
        B_c = (-0.5 / (np.exp(lv_c) + _TOL)).astype(np.float32)
        A_c = (-0.5 * (lv_c + LOG_2PI)).astype(np.float32)
        m[:, d] = (A_c + M2[:, d : d + 1] * B_c).max(axis=1)
    return A, B, M2, m


def make_in_maps(target, x_mean, z_mean, z_log_var):
    A, B, M2, m = host_prep(z_mean, z_log_var)
    t = np.ascontiguousarray(np.asarray(target, dtype=np.float32))
    xm = np.ascontiguousarray(np.asarray(x_mean, dtype=np.float32))

    ones_j = np.ones(N, dtype=np.float32)
    b1_rhs = np.zeros((128, N), dtype=np.float32)
    for d in range(NDBIG):
        b1_rhs[2 * d] = B[:, d]
        b1_rhs[2 * d + 1] = A[:, d]
    b1_rhs[126] = ones_j
    b1_rhs_t = np.stack([B[:, D - 1], A[:, D - 1], ones_j]).astype(np.float32)

    Asum = A.sum(axis=1, dtype=np.float32).astype(np.float32)
    b2_rhs = np.concatenate([B.T, Asum[None, :]], axis=0).astype(np.float32)

    in_maps = []
    for c in range(NCORES):
        r0, r1 = c * ROWS, (c + 1) * ROWS
        M2c = M2[r0:r1]  # [128, D]
        mc = m[r0:r1]  # [128, D]
        lhsT = np.zeros((128, NDBIG * 128), dtype=np.float32)
        for d in range(NDBIG):
            blk = lhsT[:, d * 128 : (d + 1) * 128]
            blk[2 * d] = M2c[:, d]
            blk[2 * d + 1] = 1.0
            blk[126] = -mc[:, d]
        lhsT_t = np.stack(
            [M2c[:, D - 1], np.ones(ROWS, np.float32), -mc[:, D - 1]]
        ).astype(np.float32)
        b2_lhsT = np.concatenate([M2c.T, np.ones((1, ROWS), np.float32)], axis=0)
        in_maps.append(
            {
                "t_rows": np.ascontiguousarray(t[r0:r1]),
                "xm_rows": np.ascontiguousarray(xm[r0:r1]),
                "b1_lhsT": lhsT,
                "b1_rhs": b1_rhs,
                "b1_lhsT_t": np.ascontiguousarray(lhsT_t),
                "b1_rhs_t": np.ascontiguousarray(b1_rhs_t),
                "b2_lhsT": np.ascontiguousarray(b2_lhsT.astype(np.float32)),
                "b2_rhs": np.ascontiguousarray(b2_rhs),
            }
        )
    return in_maps, m


def finish(results, m):
    """results: list of 8 per-core output dicts; m: [N, D] f32 host maxes."""
    S = sum(r["u_parts"].astype(np.float64).sum() for r in results)
    logS = math.log(S)
    msum = m.astype(np.float64).sum(axis=1)  # [N]
    log_qz_prod = D * (logS - LOG_NM) + msum

    m2 = -np.concatenate([r["negm2"][:, 0] for r in results]).astype(np.float64)
    S2 = sum(r["u2"].astype(np.float64).sum() for r in results)
    log_qz = math.log(S2) + m2 - LOG_NM

    log_px = (
        sum(
            r["psums"].astype(np.float64).sum() + r["l2sums"].astype(np.float64).sum()
            for r in results
        )
        / N
    )
    out = -(log_px - 5.0 * log_qz.mean() + 5.0 * log_qz_prod.mean())
    return np.asarray(out, dtype=np.float32)


def kernel(target, x_mean, x_log_var=None, z_mean=None, z_log_var=None, **_):
    nc = _get_program()
    in_maps, m = make_in_maps(target, x_mean, z_mean, z_log_var)
    res = run_bass_kernel_spmd(nc, in_maps, core_ids=list(range(NCORES)))
    return finish(res.results, m)


if __name__ == "__main__":
    # quick smoke: build the program only
    _get_program()
    print("program built ok")


# revision 6
# speedup vs baseline: 1.3838x; 1.3838x over previous
"""Beta-TCVAE loss kernel for Trainium2, 8 NeuronCores, data-parallel over rows.

Math (see reference): with elem[i,j,d] = A[j,d] + M2[i,d]*B[j,d] where
  A = -0.5*(zlv + log 2pi), B = -0.5/(exp(zlv)+tol), M2 = z_mean^2,
the loss collapses (log_pz cancels exactly) to
  out = -(log_px - 5*mean_i log_qz[i] + 5*mean_i log_qz_prod[i])
  log_qz_prod[i] = D*(log S - log nm) + sum_d m[i,d],
      m[i,d] = max_j elem[i,j,d],  S = sum_{i,j,d} exp(elem - m[i,d])
  log_qz[i] = log S2 + m2[i] - log nm,
      R[i,j] = Asum[j] + sum_d M2[i,d]B[j,d],  m2[i] = max_j R,
      S2 = sum_{i,j} exp(R - m2[i])
  log_px = mean_i sum_p [t*log(xm+tol) + (1-t)*log(1-xm+tol)]

m[i,d] is computed EXACTLY on host: elem as a function of lv = zlv[j,d] is
strictly concave, so the discrete max over j lies at the sorted-lv values
bracketing the continuous argmax (u* solves x*u = (u+tol)^2).  All
O(N^2 D) / O(N PIX) work runs on the device:
 - TensorE forms (elem - m) via K=128 matmuls whose zero-padded bf16
   weights carry, per d, 7 rows: the hi/lo split products
   {M2hi*Bhi, M2hi*Blo, M2lo*Bhi}, {1*Ahi, 1*Alo}, {(-m)hi*1, (-m)lo*1}
   (bf16 hi+lo keeps |elem - m| accurate to ~5e-4; fp32 matmul would
   lower to 2x instructions and dominate the kernel).
 - ScalarE does exp with fused accumulation straight out of PSUM.
 - log_px: ScalarE Ln (x2) + VectorE sub + fused multiply-accum-reduce.
ScalarE table thrash (Ln vs Exp sets) is avoided by running all exps
first and gating the Ln bias tiles on the exp outputs.
Per-core partial sums return to host; final combination in float64.
"""

import math

import ml_dtypes
import numpy as np

import concourse.bacc as bacc
import concourse.tile as tile
from concourse import mybir
from concourse.bass_utils import run_bass_kernel_spmd

F32 = mybir.dt.float32
BF16 = mybir.dt.bfloat16
AF = mybir.ActivationFunctionType
ALU = mybir.AluOpType
NP_BF16 = ml_dtypes.bfloat16

_TOL = 1e-7
DATASET_SIZE = 737280
N, D, PIX = 1024, 64, 12288
LOG_2PI = math.log(2.0 * math.pi)
LOG_NM = math.log(float(N * DATASET_SIZE))
NCORES = 8
ROWS = N // NCORES  # 128
CH = 2048
NCH = PIX // CH  # 6
DPAIRS = D // 2  # 32 psum tiles, 2 d's each
RPD = 7  # lhsT/rhs rows per d (3 product rows + 2 A rows + 2 m rows)
DPP = 18  # d's per pack (7*18 = 126 <= 128)
NPACK = (D + DPP - 1) // DPP  # 4 (18+18+18+10)


def _pack_dcount(p):
    return min(DPP, D - p * DPP)


def _build_program():
    nc = bacc.Bacc("TRN2", target_bir_lowering=False, debug=False)

    # ---- DRAM I/O (per core; SPMD over 8 cores) ----
    t_rows = nc.dram_tensor("t_rows", [ROWS, PIX], F32, kind="ExternalInput")
    xm_rows = nc.dram_tensor("xm_rows", [ROWS, PIX], F32, kind="ExternalInput")
    lhsT_d = [
        nc.dram_tensor(f"b1_lhsT_{p}", [128, _pack_dcount(p) * 128], BF16, kind="ExternalInput")
        for p in range(NPACK)
    ]
    rhs_d = [
        nc.dram_tensor(f"b1_rhs_{p}", [128, N], BF16, kind="ExternalInput")
        for p in range(NPACK)
    ]
    b2_lhsT = nc.dram_tensor("b2_lhsT", [D + 1, 128], F32, kind="ExternalInput")
    b2_rhs = nc.dram_tensor("b2_rhs", [D + 1, N], F32, kind="ExternalInput")

    u_parts_d = nc.dram_tensor("u_parts", [128, DPAIRS], F32, kind="ExternalOutput")
    negm2_d = nc.dram_tensor("negm2", [128, 1], F32, kind="ExternalOutput")
    u2_d = nc.dram_tensor("u2", [128, 1], F32, kind="ExternalOutput")
    l2sums_d = nc.dram_tensor("l2sums", [128, NCH], F32, kind="ExternalOutput")
    psums_d = nc.dram_tensor("psums", [128, NCH], F32, kind="ExternalOutput")

    with tile.TileContext(nc) as tc:
        with (
            tc.tile_pool(name="consts", bufs=1) as consts,
            tc.tile_pool(name="chunks", bufs=NCH) as chunks,
            tc.tile_pool(name="lnp", bufs=2) as lnp,
            tc.tile_pool(name="scr", bufs=2) as scr,
            tc.tile_pool(name="outs", bufs=1) as outs,
            tc.tile_pool(name="psum", bufs=2, space="PSUM") as psum,
        ):
            # resident small tensors (emitted first so PE can start early)
            lhsT_s = []
            rhs_s = []
            for p in range(NPACK):
                lt = consts.tile([128, _pack_dcount(p) * 128], BF16, tag=f"l{p}")
                nc.sync.dma_start(out=lt, in_=lhsT_d[p][:, :])
                lhsT_s.append(lt)
                rt = consts.tile([128, N], BF16, tag=f"r{p}")
                nc.sync.dma_start(out=rt, in_=rhs_d[p][:, :])
                rhs_s.append(rt)
            b2_lhsT_s = consts.tile([D + 1, 128], F32, tag="b2l")
            nc.sync.dma_start(out=b2_lhsT_s, in_=b2_lhsT[:, :])
            b2_rhs_s = consts.tile([D + 1, N], F32, tag="b2r")
            nc.sync.dma_start(out=b2_rhs_s, in_=b2_rhs[:, :])

            tol_c = consts.tile([128, 1], F32, tag="tc")
            nc.vector.memset(tol_c, _TOL)
            onep_c = consts.tile([128, 1], F32, tag="oc")
            nc.vector.memset(onep_c, 1.0 + _TOL)
            zero_c = consts.tile([128, 1], F32, tag="zc")
            nc.vector.memset(zero_c, 0.0)

            u_parts_s = outs.tile([128, DPAIRS], F32)
            negm2_s = outs.tile([128, 1], F32)
            u2_s = outs.tile([128, 1], F32)
            l2sums_s = outs.tile([128, NCH], F32)
            psums_s = outs.tile([128, NCH], F32)
            tol_gate = outs.tile([128, DPAIRS], F32)
            onep_gate = outs.tile([128, DPAIRS], F32)

            # ---- B2: R = Asum[j] + sum_d M2[i,d] B[j,d]; m2, U2 (fp32) ----
            r_ps = psum.tile([128, N], F32, tag="pt")
            for j0 in (0, 512):
                nc.tensor.matmul(
                    out=r_ps[:, j0 : j0 + 512],
                    lhsT=b2_lhsT_s,
                    rhs=b2_rhs_s[:, j0 : j0 + 512],
                    start=True,
                    stop=True,
                )
            nc.vector.tensor_reduce(
                out=negm2_s,
                in_=r_ps,
                axis=mybir.AxisListType.X,
                op=ALU.max,
                negate=True,
            )
            nc.scalar.activation(
                out=r_ps,
                in_=r_ps,
                func=AF.Exp,
                bias=negm2_s[:],
                scale=1.0,
                accum_out=u2_s,
            )
            nc.sync.dma_start(out=negm2_d[:, :], in_=negm2_s)
            nc.sync.dma_start(out=u2_d[:, :], in_=u2_s)

            # ---- B1: 32 psum tiles, each holds (elem - m) for 2 d's ----
            for k in range(DPAIRS):
                pt = psum.tile([128, 2 * N], F32, tag="pt")
                for half in range(2):
                    d = 2 * k + half
                    p, t = d // DPP, d % DPP
                    for j0 in (0, 512):
                        nc.tensor.matmul(
                            out=pt[:, half * N + j0 : half * N + j0 + 512],
                            lhsT=lhsT_s[p][:, t * 128 : (t + 1) * 128],
                            rhs=rhs_s[p][:, j0 : j0 + 512],
                            start=True,
                            stop=True,
                        )
                nc.scalar.activation(
                    out=pt,
                    in_=pt,
                    func=AF.Exp,
                    bias=zero_c[:],
                    scale=1.0,
                    accum_out=u_parts_s[:, k : k + 1],
                )
            nc.sync.dma_start(out=u_parts_d[:, :], in_=u_parts_s)

            # ---- gates: ACT-side bias tiles that depend on every exp ----
            # (forces all Ln instructions after all Exp instructions ->
            #  exactly two ACT table loads instead of per-switch thrash)
            nc.scalar.activation(
                out=tol_gate, in_=u_parts_s, func=AF.Identity, bias=tol_c[:], scale=0.0
            )
            nc.scalar.activation(
                out=onep_gate, in_=u_parts_s, func=AF.Identity, bias=onep_c[:], scale=0.0
            )

            # ---- A: log_px partial sums ----
            for c in range(NCH):
                tt = chunks.tile([128, CH], F32, tag="tt")
                nc.sync.dma_start(out=tt, in_=t_rows[:, c * CH : (c + 1) * CH])
                xt = chunks.tile([128, CH], F32, tag="xt")
                nc.sync.dma_start(out=xt, in_=xm_rows[:, c * CH : (c + 1) * CH])
                l1 = lnp.tile([128, CH], F32, tag="l1")
                nc.scalar.activation(
                    out=l1, in_=xt, func=AF.Ln, bias=tol_gate[:, 0:1], scale=1.0
                )
                l2 = lnp.tile([128, CH], F32, tag="l2")
                nc.scalar.activation(
                    out=l2,
                    in_=xt,
                    func=AF.Ln,
                    bias=onep_gate[:, 0:1],
                    scale=-1.0,
                    accum_out=l2sums_s[:, c : c + 1],
                )
                nc.vector.tensor_sub(out=l1, in0=l1, in1=l2)
                ps = scr.tile([128, CH], F32, tag="ps")
                nc.vector.scalar_tensor_tensor(
                    out=ps,
                    in0=tt,
                    scalar=1.0,
                    in1=l1,
                    op0=ALU.mult,
                    op1=ALU.mult,
                    accum_out=psums_s[:, c : c + 1],
                )
            nc.sync.dma_start(out=l2sums_d[:, :], in_=l2sums_s)
            nc.sync.dma_start(out=psums_d[:, :], in_=psums_s)

    nc.compile()
    return nc


_NC_CACHE = None


def _get_program():
    global _NC_CACHE
    if _NC_CACHE is None:
        _NC_CACHE = _build_program()
    return _NC_CACHE


def host_prep(z_mean, z_log_var):
    """A, B, M2 [N,D] f32 and the exact per-(i,d) max m [N,D] f32."""
    zlv = np.asarray(z_log_var, dtype=np.float32)
    M2 = np.square(np.asarray(z_mean, dtype=np.float32))
    ez = np.exp(zlv)
    B = (-0.5 / (ez + _TOL)).astype(np.float32)
    A = (-0.5 * (zlv + LOG_2PI)).astype(np.float32)

    x = M2.astype(np.float64)
    tol = float(_TOL)
    disc = np.maximum((x - 2 * tol) ** 2 - 4 * tol * tol, 0.0)
    ustar = ((x - 2 * tol) + np.sqrt(disc)) / 2.0
    with np.errstate(divide="ignore"):
        lvstar = np.where(x <= 4 * tol, -np.inf, np.log(np.maximum(ustar, 1e-300)))

    m = np.empty((N, D), dtype=np.float32)
    for d in range(D):
        s = np.sort(zlv[:, d].astype(np.float64))
        pos = np.searchsorted(s, lvstar[:, d])
        cands = np.stack([np.clip(pos + k, 0, N - 1) for k in (-2, -1, 0, 1)], axis=1)
        lv_c = s[cands].astype(np.float32)
        B_c = (-0.5 / (np.exp(lv_c) + _TOL)).astype(np.float32)
        A_c = (-0.5 * (lv_c + LOG_2PI)).astype(np.float32)
        m[:, d] = (A_c + M2[:, d : d + 1] * B_c).max(axis=1)
    return A, B, M2, m


def _split(x):
    """bf16 hi/lo split: x ~= hi + lo with both bf16."""
    hi = x.astype(NP_BF16)
    lo = (x.astype(np.float32) - hi.astype(np.float32)).astype(NP_BF16)
    return hi, lo


def make_in_maps(target, x_mean, z_mean, z_log_var):
    A, B, M2, m = host_prep(z_mean, z_log_var)
    t = np.ascontiguousarray(np.asarray(target, dtype=np.float32))
    xm = np.ascontiguousarray(np.asarray(x_mean, dtype=np.float32))

    B_hi, B_lo = _split(B)  # [N, D]
    A_hi, A_lo = _split(A)
    ones_j = np.ones(N, dtype=NP_BF16)

    # shared rhs packs [128, N] bf16: rows 7t.. = Bhi, Blo, Bhi, Ahi, Alo, 1, 1
    rhs_packs = []
    for p in range(NPACK):
        nd = _pack_dcount(p)
        R = np.zeros((128, N), dtype=NP_BF16)
        for tt in range(nd):
            d = p * DPP + tt
            r = RPD * tt
            R[r + 0] = B_hi[:, d]
            R[r + 1] = B_lo[:, d]
            R[r + 2] = B_hi[:, d]
            R[r + 3] = A_hi[:, d]
            R[r + 4] = A_lo[:, d]
            R[r + 5] = ones_j
            R[r + 6] = ones_j
        rhs_packs.append(R)

    Asum = A.sum(axis=1, dtype=np.float32).astype(np.float32)
    b2_rhs = np.ascontiguousarray(
        np.concatenate([B.T, Asum[None, :]], axis=0).astype(np.float32)
    )

    in_maps = []
    for c in range(NCORES):
        r0, r1 = c * ROWS, (c + 1) * ROWS
        M2_hi, M2_lo = _split(M2[r0:r1])  # [128, D]
        nm_hi, nm_lo = _split(-m[r0:r1])
        ones_i = np.ones(ROWS, dtype=NP_BF16)
        im = {
            "t_rows": np.ascontiguousarray(t[r0:r1]),
            "xm_rows": np.ascontiguousarray(xm[r0:r1]),
            "b2_lhsT": np.ascontiguousarray(
                np.concatenate(
                    [M2[r0:r1].T, np.ones((1, ROWS), np.float32)], axis=0
                ).astype(np.float32)
            ),
            "b2_rhs": b2_rhs,
        }
        for p in range(NPACK):
            nd = _pack_dcount(p)
            L = np.zeros((128, nd * 128), dtype=NP_BF16)
            for tt in range(nd):
                d = p * DPP + tt
                blk = L[:, tt * 128 : (tt + 1) * 128]
                r = RPD * tt
                blk[r + 0] = M2_hi[:, d]
                blk[r + 1] = M2_hi[:, d]
                blk[r + 2] = M2_lo[:, d]
                blk[r + 3] = ones_i
                blk[r + 4] = ones_i
                blk[r + 5] = nm_hi[:, d]
                blk[r + 6] = nm_lo[:, d]
            im[f"b1_lhsT_{p}"] = L
            im[f"b1_rhs_{p}"] = rhs_packs[p]
        in_maps.append(im)
    return in_maps, m


def finish(results, m):
    """results: list of 8 per-core output dicts; m: [N, D] f32 host maxes."""
    S = sum(r["u_parts"].astype(np.float64).sum() for r in results)
    logS = math.log(S)
    msum = m.astype(np.float64).sum(axis=1)  # [N]
    log_qz_prod = D * (logS - LOG_NM) + msum

    m2 = -np.concatenate([r["negm2"][:, 0] for r in results]).astype(np.float64)
    S2 = sum(r["u2"].astype(np.float64).sum() for r in results)
    log_qz = math.log(S2) + m2 - LOG_NM

    log_px = (
        sum(
            r["psums"].astype(np.float64).sum() + r["l2sums"].astype(np.float64).sum()
            for r in results
        )
        / N
    )
    out = -(log_px - 5.0 * log_qz.mean() + 5.0 * log_qz_prod.mean())
    return np.asarray(out, dtype=np.float32)


def kernel(target, x_mean, x_log_var=None, z_mean=None, z_log_var=None, **_):
    nc = _get_program()
    in_maps, m = make_in_maps(target, x_mean, z_mean, z_log_var)
    res = run_bass_kernel_spmd(nc, in_maps, core_ids=list(range(NCORES)))
    return finish(res.results, m)


if __name__ == "__main__":
    _get_program()
    print("program built ok")


# revision 7
# speedup vs baseline: 1.4597x; 1.0549x over previous
"""Beta-TCVAE loss kernel for Trainium2, 8 NeuronCores, data-parallel over rows.

Math (see reference): with elem[i,j,d] = A[j,d] + M2[i,d]*B[j,d] where
  A = -0.5*(zlv + log 2pi), B = -0.5/(exp(zlv)+tol), M2 = z_mean^2,
the loss collapses (log_pz cancels exactly) to
  out = -(log_px - 5*mean_i log_qz[i] + 5*mean_i log_qz_prod[i])
  log_qz_prod[i] = D*(log S - log nm) + sum_d m[i,d],
      m[i,d] = max_j elem[i,j,d],  S = sum_{i,j,d} exp(elem - m[i,d])
  log_qz[i] = log S2 + m2[i] - log nm,
      R[i,j] = Asum[j] + sum_d M2[i,d]B[j,d],  m2[i] = max_j R,
      S2 = sum_{i,j} exp(R - m2[i])
  log_px = mean_i sum_p [t*log(xm+tol) + (1-t)*log(1-xm+tol)]

m[i,d] is computed EXACTLY on host: elem as a function of lv = zlv[j,d] is
strictly concave, so the discrete max over j lies at the sorted-lv values
bracketing the continuous argmax (u* solves x*u = (u+tol)^2).  All
O(N^2 D) / O(N PIX) work runs on the device:
 - TensorE forms (elem - m) via K=128 matmuls whose zero-padded bf16
   weights carry, per d, 7 rows: the hi/lo split products
   {M2hi*Bhi, M2hi*Blo, M2lo*Bhi}, {1*Ahi, 1*Alo}, {(-m)hi*1, (-m)lo*1}
   (bf16 hi+lo keeps |elem - m| accurate to ~5e-4; fp32 matmul would
   lower to 2x instructions and dominate the kernel).
 - ScalarE does exp with fused accumulation straight out of PSUM.
 - log_px: ScalarE Ln (x2) + VectorE sub + fused multiply-accum-reduce.
ScalarE table thrash (Ln vs Exp sets) is avoided by running all exps
first and gating the Ln bias tiles on the exp outputs.
Per-core partial sums return to host; final combination in float64.
"""

import math

import ml_dtypes
import numpy as np

import concourse.bacc as bacc
import concourse.tile as tile
from concourse import mybir
from concourse.bass_utils import run_bass_kernel_spmd

F32 = mybir.dt.float32
BF16 = mybir.dt.bfloat16
AF = mybir.ActivationFunctionType
ALU = mybir.AluOpType
NP_BF16 = ml_dtypes.bfloat16

_TOL = 1e-7
DATASET_SIZE = 737280
N, D, PIX = 1024, 64, 12288
LOG_2PI = math.log(2.0 * math.pi)
LOG_NM = math.log(float(N * DATASET_SIZE))
NCORES = 8
ROWS = N // NCORES  # 128
CH = 2048
NCH = PIX // CH  # 6
DPAIRS = D // 2  # 32 psum tiles, 2 d's each
RPD = 7  # lhsT/rhs rows per d (3 product rows + 2 A rows + 2 m rows)
DPP = 18  # d's per pack (7*18 = 126 <= 128)
NPACK = (D + DPP - 1) // DPP  # 4 (18+18+18+10)


def _pack_dcount(p):
    return min(DPP, D - p * DPP)


def _build_program():
    nc = bacc.Bacc("TRN2", target_bir_lowering=False, debug=False)

    # ---- DRAM I/O (per core; SPMD over 8 cores) ----
    t_rows = nc.dram_tensor("t_rows", [ROWS, PIX], F32, kind="ExternalInput")
    xm_rows = nc.dram_tensor("xm_rows", [ROWS, PIX], F32, kind="ExternalInput")
    lhsT_d = [
        nc.dram_tensor(f"b1_lhsT_{p}", [128, _pack_dcount(p) * 128], BF16, kind="ExternalInput")
        for p in range(NPACK)
    ]
    rhs_d = [
        nc.dram_tensor(f"b1_rhs_{p}", [128, N], BF16, kind="ExternalInput")
        for p in range(NPACK)
    ]
    b2_lhsT = nc.dram_tensor("b2_lhsT", [D + 1, 128], F32, kind="ExternalInput")
    b2_rhs = nc.dram_tensor("b2_rhs", [D + 1, N], F32, kind="ExternalInput")

    u_parts_d = nc.dram_tensor("u_parts", [128, DPAIRS], F32, kind="ExternalOutput")
    negm2_d = nc.dram_tensor("negm2", [128, 1], F32, kind="ExternalOutput")
    u2_d = nc.dram_tensor("u2", [128, 1], F32, kind="ExternalOutput")
    l2sums_d = nc.dram_tensor("l2sums", [128, NCH], F32, kind="ExternalOutput")
    psums_d = nc.dram_tensor("psums", [128, NCH], F32, kind="ExternalOutput")

    with tile.TileContext(nc) as tc:
        with (
            tc.tile_pool(name="consts", bufs=1) as consts,
            tc.tile_pool(name="chunks", bufs=NCH) as chunks,
            tc.tile_pool(name="lnp", bufs=2) as lnp,
            tc.tile_pool(name="scr", bufs=2) as scr,
            tc.tile_pool(name="outs", bufs=1) as outs,
            tc.tile_pool(name="psum", bufs=2, space="PSUM") as psum,
        ):
            # resident small tensors (emitted first so PE can start early)
            lhsT_s = []
            rhs_s = []
            for p in range(NPACK):
                lt = consts.tile([128, _pack_dcount(p) * 128], BF16, tag=f"l{p}")
                nc.sync.dma_start(out=lt, in_=lhsT_d[p][:, :])
                lhsT_s.append(lt)
                rt = consts.tile([128, N], BF16, tag=f"r{p}")
                nc.sync.dma_start(out=rt, in_=rhs_d[p][:, :])
                rhs_s.append(rt)
            b2_lhsT_s = consts.tile([D + 1, 128], F32, tag="b2l")
            nc.sync.dma_start(out=b2_lhsT_s, in_=b2_lhsT[:, :])
            b2_rhs_s = consts.tile([D + 1, N], F32, tag="b2r")
            nc.sync.dma_start(out=b2_rhs_s, in_=b2_rhs[:, :])

            zero_c = consts.tile([128, 1], F32, tag="zc")
            nc.vector.memset(zero_c, 0.0)

            u_parts_s = outs.tile([128, DPAIRS], F32)
            negm2_s = outs.tile([128, 1], F32)
            u2_s = outs.tile([128, 1], F32)
            l2sums_s = outs.tile([128, NCH], F32)
            psums_s = outs.tile([128, NCH], F32)
            tol_gate = outs.tile([128, DPAIRS], F32)
            onep_gate = outs.tile([128, DPAIRS], F32)

            # ---- B1: 32 psum tiles, each holds (elem - m) for 2 d's ----
            for k in range(DPAIRS):
                pt = psum.tile([128, 2 * N], F32, tag="pt")
                for half in range(2):
                    d = 2 * k + half
                    p, t = d // DPP, d % DPP
                    for j0 in (0, 512):
                        nc.tensor.matmul(
                            out=pt[:, half * N + j0 : half * N + j0 + 512],
                            lhsT=lhsT_s[p][:, t * 128 : (t + 1) * 128],
                            rhs=rhs_s[p][:, j0 : j0 + 512],
                            start=True,
                            stop=True,
                        )
                nc.scalar.activation(
                    out=pt,
                    in_=pt,
                    func=AF.Exp,
                    bias=zero_c[:],
                    scale=1.0,
                    accum_out=u_parts_s[:, k : k + 1],
                )
            nc.sync.dma_start(out=u_parts_d[:, :], in_=u_parts_s)

            # ---- B2: R = Asum[j] + sum_d M2[i,d] B[j,d]; m2, U2 (fp32) ----
            r_ps = psum.tile([128, N], F32, tag="pt")
            for j0 in (0, 512):
                nc.tensor.matmul(
                    out=r_ps[:, j0 : j0 + 512],
                    lhsT=b2_lhsT_s,
                    rhs=b2_rhs_s[:, j0 : j0 + 512],
                    start=True,
                    stop=True,
                )
            nc.vector.tensor_reduce(
                out=negm2_s,
                in_=r_ps,
                axis=mybir.AxisListType.X,
                op=ALU.max,
                negate=True,
            )
            nc.scalar.activation(
                out=r_ps,
                in_=r_ps,
                func=AF.Exp,
                bias=negm2_s[:],
                scale=1.0,
                accum_out=u2_s,
            )
            nc.sync.dma_start(out=negm2_d[:, :], in_=negm2_s)
            nc.sync.dma_start(out=u2_d[:, :], in_=u2_s)

            # ---- gates: ACT-side bias tiles that depend on every exp ----
            # (forces all Ln instructions after all Exp instructions ->
            #  exactly two ACT table loads instead of per-switch thrash)
            tol_c2 = consts.tile([128, 1], F32, tag="tc2")
            nc.vector.tensor_scalar(
                out=tol_c2, in0=u2_s, scalar1=0.0, scalar2=_TOL,
                op0=ALU.mult, op1=ALU.add,
            )
            onep_c2 = consts.tile([128, 1], F32, tag="oc2")
            nc.vector.tensor_scalar(
                out=onep_c2, in0=u2_s, scalar1=0.0, scalar2=1.0 + _TOL,
                op0=ALU.mult, op1=ALU.add,
            )
            nc.scalar.activation(
                out=tol_gate, in_=u_parts_s, func=AF.Identity, bias=tol_c2[:], scale=0.0
            )
            nc.scalar.activation(
                out=onep_gate, in_=u_parts_s, func=AF.Identity, bias=onep_c2[:], scale=0.0
            )

            # ---- A: log_px partial sums ----
            for c in range(NCH):
                tt = chunks.tile([128, CH], F32, tag="tt")
                nc.gpsimd.dma_start(out=tt, in_=t_rows[:, c * CH : (c + 1) * CH])
                xt = chunks.tile([128, CH], F32, tag="xt")
                nc.gpsimd.dma_start(out=xt, in_=xm_rows[:, c * CH : (c + 1) * CH])
                l1 = lnp.tile([128, CH], F32, tag="l1")
                nc.scalar.activation(
                    out=l1, in_=xt, func=AF.Ln, bias=tol_gate[:, 0:1], scale=1.0
                )
                l2 = lnp.tile([128, CH], F32, tag="l2")
                nc.scalar.activation(
                    out=l2,
                    in_=xt,
                    func=AF.Ln,
                    bias=onep_gate[:, 0:1],
                    scale=-1.0,
                    accum_out=l2sums_s[:, c : c + 1],
                )
                nc.vector.tensor_sub(out=l1, in0=l1, in1=l2)
                ps = scr.tile([128, CH], F32, tag="ps")
                nc.vector.scalar_tensor_tensor(
                    out=ps,
                    in0=tt,
                    scalar=1.0,
                    in1=l1,
                    op0=ALU.mult,
                    op1=ALU.mult,
                    accum_out=psums_s[:, c : c + 1],
                )
            nc.sync.dma_start(out=l2sums_d[:, :], in_=l2sums_s)
            nc.sync.dma_start(out=psums_d[:, :], in_=psums_s)

    nc.compile()
    return nc


_NC_CACHE = None


def _get_program():
    global _NC_CACHE
    if _NC_CACHE is None:
        _NC_CACHE = _build_program()
    return _NC_CACHE


def host_prep(z_mean, z_log_var):
    """A, B, M2 [N,D] f32 and the exact per-(i,d) max m [N,D] f32."""
    zlv = np.asarray(z_log_var, dtype=np.float32)
    M2 = np.square(np.asarray(z_mean, dtype=np.float32))
    ez = np.exp(zlv)
    B = (-0.5 / (ez + _TOL)).astype(np.float32)
    A = (-0.5 * (zlv + LOG_2PI)).astype(np.float32)

    x = M2.astype(np.float64)
    tol = float(_TOL)
    disc = np.maximum((x - 2 * tol) ** 2 - 4 * tol * tol, 0.0)
    ustar = ((x - 2 * tol) + np.sqrt(disc)) / 2.0
    with np.errstate(divide="ignore"):
        lvstar = np.where(x <= 4 * tol, -np.inf, np.log(np.maximum(ustar, 1e-300)))

    m = np.empty((N, D), dtype=np.float32)
    for d in range(D):
        s = np.sort(zlv[:, d].astype(np.float64))
        pos = np.searchsorted(s, lvstar[:, d])
        cands = np.stack([np.clip(pos + k, 0, N - 1) for k in (-2, -1, 0, 1)], axis=1)
        lv_c = s[cands].astype(np.float32)
        B_c = (-0.5 / (np.exp(lv_c) + _TOL)).astype(np.float32)
        A_c = (-0.5 * (lv_c + LOG_2PI)).astype(np.float32)
        m[:, d] = (A_c + M2[:, d : d + 1] * B_c).max(axis=1)
    return A, B, M2, m


def _split(x):
    """bf16 hi/lo split: x ~= hi + lo with both bf16."""
    hi = x.astype(NP_BF16)
    lo = (x.astype(np.float32) - hi.astype(np.float32)).astype(NP_BF16)
    return hi, lo


def make_in_maps(target, x_mean, z_mean, z_log_var):
    A, B, M2, m = host_prep(z_mean, z_log_var)
    t = np.ascontiguousarray(np.asarray(target, dtype=np.float32))
    xm = np.ascontiguousarray(np.asarray(x_mean, dtype=np.float32))

    B_hi, B_lo = _split(B)  # [N, D]
    A_hi, A_lo = _split(A)
    ones_j = np.ones(N, dtype=NP_BF16)

    # shared rhs packs [128, N] bf16: rows 7t.. = Bhi, Blo, Bhi, Ahi, Alo, 1, 1
    rhs_packs = []
    for p in range(NPACK):
        nd = _pack_dcount(p)
        R = np.zeros((128, N), dtype=NP_BF16)
        for tt in range(nd):
            d = p * DPP + tt
            r = RPD * tt
            R[r + 0] = B_hi[:, d]
            R[r + 1] = B_lo[:, d]
            R[r + 2] = B_hi[:, d]
            R[r + 3] = A_hi[:, d]
            R[r + 4] = A_lo[:, d]
            R[r + 5] = ones_j
            R[r + 6] = ones_j
        rhs_packs.append(R)

    Asum = A.sum(axis=1, dtype=np.float32).astype(np.float32)
    b2_rhs = np.ascontiguousarray(
        np.concatenate([B.T, Asum[None, :]], axis=0).astype(np.float32)
    )

    in_maps = []
    for c in range(NCORES):
        r0, r1 = c * ROWS, (c + 1) * ROWS
        M2_hi, M2_lo = _split(M2[r0:r1])  # [128, D]
        nm_hi, nm_lo = _split(-m[r0:r1])
        ones_i = np.ones(ROWS, dtype=NP_BF16)
        im = {
            "t_rows": np.ascontiguousarray(t[r0:r1]),
            "xm_rows": np.ascontiguousarray(xm[r0:r1]),
            "b2_lhsT": np.ascontiguousarray(
                np.concatenate(
                    [M2[r0:r1].T, np.ones((1, ROWS), np.float32)], axis=0
                ).astype(np.float32)
            ),
            "b2_rhs": b2_rhs,
        }
        for p in range(NPACK):
            nd = _pack_dcount(p)
            L = np.zeros((128, nd * 128), dtype=NP_BF16)
            for tt in range(nd):
                d = p * DPP + tt
                blk = L[:, tt * 128 : (tt + 1) * 128]
                r = RPD * tt
                blk[r + 0] = M2_hi[:, d]
                blk[r + 1] = M2_hi[:, d]
                blk[r + 2] = M2_lo[:, d]
                blk[r + 3] = ones_i
                blk[r + 4] = ones_i
                blk[r + 5] = nm_hi[:, d]
                blk[r + 6] = nm_lo[:, d]
            im[f"b1_lhsT_{p}"] = L
            im[f"b1_rhs_{p}"] = rhs_packs[p]
        in_maps.append(im)
    return in_maps, m


def finish(results, m):
    """results: list of 8 per-core output dicts; m: [N, D] f32 host maxes."""
    S = sum(r["u_parts"].astype(np.float64).sum() for r in results)
    logS = math.log(S)
    msum = m.astype(np.float64).sum(axis=1)  # [N]
    log_qz_prod = D * (logS - LOG_NM) + msum

    m2 = -np.concatenate([r["negm2"][:, 0] for r in results]).astype(np.float64)
    S2 = sum(r["u2"].astype(np.float64).sum() for r in results)
    log_qz = math.log(S2) + m2 - LOG_NM

    log_px = (
        sum(
            r["psums"].astype(np.float64).sum() + r["l2sums"].astype(np.float64).sum()
            for r in results
        )
        / N
    )
    out = -(log_px - 5.0 * log_qz.mean() + 5.0 * log_qz_prod.mean())
    return np.asarray(out, dtype=np.float32)


def kernel(target, x_mean, x_log_var=None, z_mean=None, z_log_var=None, **_):
    nc = _get_program()
    in_maps, m = make_in_maps(target, x_mean, z_mean, z_log_var)
    res = run_bass_kernel_spmd(nc, in_maps, core_ids=list(range(NCORES)))
    return finish(res.results, m)


if __name__ == "__main__":
    _get_program()
    print("program built ok")
